# revision 7
# baseline (speedup 1.0000x reference)
"""Trainium2 Bass kernel for nn_DWIAngleLinear (A-Softmax / vq_codebook).

Math (see reference):
  out      = (one_hot*(phi-cos)/(1+lamb) + cos) * ||x||    [256, 85742]
  new_w    = scatter-mean(f_norm, label) over touched rows  [85742, 512]

Key restructuring for the device:
  * Bulk of `out` is cos_theta * ||x||  =  x @ (W / ||W||_rows).T  -- using the
    UNNORMALIZED input folds the ||x|| row scale into the matmul for free.
  * The one-hot margin term touches only 256 elements -> host fixup.
  * new_weight differs from weight in <=256 rows -> device streams the weight
    shard back out; host patches the touched rows with the scatter-means.

Sharding: classes (out_features) split across 8 cores, padded to 8 x 10752.
Each core: W shard [10752, 512] + x^T [512, 256] -> out shard [256, 10752]
and new_weight shard [10752, 512].

Per 128-class chunk on each core:
  DMA W chunk (class-major) -> sumsq (ACT/DVE alternating) -> 1/||w|| via
  DVE-reciprocal + ACT-sqrt + 2 Newton steps (batched per 12 chunks) ->
  normalize (DVE tensor_scalar) -> PE-transpose to feat-major -> 8 fp32
  matmuls vs resident x^T -> PSUM->SBUF copy into a staged [256, S] buffer
  -> bulk DMA out.  W chunk also DMA'd back out as the new_weight shard.
"""

import numpy as np

BS = 256
IN_F = 512
OUT_F = 85742
NCORES = 8
P = 128
S_PAD = 10752            # classes per core, = CHUNKS * P
CHUNKS = 84
GROUP = 6                # chunks per rsqrt batch (divides CHUNKS)
OUT_DMA_GROUP = 21       # chunks per output-column DMA flush (divides CHUNKS)

M_MARGIN = 4
PI_APPROX = 3.14159265
LAMB = max(5.0, 1000.0 * (1.0 + 0.12 * 1) ** (-1))
EPS = 1e-12

_COMPILED_NC = None


def _build_nc(variant="b_fp32"):
    """Build + compile the per-core Bass/Tile kernel (label independent).

    variant:
      "b_fp32"  - batch-major matmul output, full fp32 matmuls (4 cyc/row)
      "a_fp32r" - class-major matmul output (N=256 moving dim), TF32 matmuls
    """
    from contextlib import ExitStack

    import concourse.bacc as bacc
    import concourse.mybir as mybir
    import concourse.tile as tile
    from concourse.masks import make_identity

    f32 = mybir.dt.float32
    f32r = mybir.dt.float32r
    mult = mybir.AluOpType.mult
    add = mybir.AluOpType.add
    subtract = mybir.AluOpType.subtract
    Square = mybir.ActivationFunctionType.Square

    nc = bacc.Bacc("TRN2", target_bir_lowering=False, debug=False)

    w = nc.dram_tensor("w", [S_PAD, IN_F], f32, kind="ExternalInput").ap()
    xT = nc.dram_tensor("xT", [IN_F, BS], f32, kind="ExternalInput").ap()
    out = nc.dram_tensor("out", [BS, S_PAD], f32, kind="ExternalOutput").ap()
    nw = nc.dram_tensor("nw", [S_PAD, IN_F], f32, kind="ExternalOutput").ap()

    with tile.TileContext(nc) as tc, ExitStack() as ctx:
        const_pool = ctx.enter_context(tc.tile_pool(name="const", bufs=1))
        outbuf_pool = ctx.enter_context(tc.tile_pool(name="outbuf", bufs=1))
        wc_pool = ctx.enter_context(tc.tile_pool(name="wc", bufs=2 * GROUP + 2))
        wn_pool = ctx.enter_context(tc.tile_pool(name="wn", bufs=3))
        sq_pool = ctx.enter_context(tc.tile_pool(name="sq", bufs=4))
        wt_pool = ctx.enter_context(tc.tile_pool(name="wt", bufs=3))
        nr_pool = ctx.enter_context(tc.tile_pool(name="nr", bufs=2))
        psa_pool = ctx.enter_context(tc.tile_pool(name="psa", bufs=2, space="PSUM"))
        psb_pool = ctx.enter_context(tc.tile_pool(name="psb", bufs=3, space="PSUM"))
        if variant == "a_fp32r":
            oc_pool = ctx.enter_context(tc.tile_pool(name="oc", bufs=3))
            psc_pool = ctx.enter_context(tc.tile_pool(name="psc", bufs=2, space="PSUM"))

        identity = const_pool.tile([P, P], f32)
        make_identity(nc, identity)

        # x^T staged as [128 feat, 4 kchunk, 256 batch]
        xt_sb = const_pool.tile([P, 4, BS], f32)
        nc.sync.dma_start(xt_sb[:], xT.rearrange("(k p) m -> p k m", p=P))

        ssq = const_pool.tile([P, CHUNKS], f32)   # per-class sum(W^2)
        invn = const_pool.tile([P, CHUNKS], f32)  # per-class 1/||W||

        # final batch-major output staged in SBUF: [128, 2 (m-half), S_PAD]
        out_sb = outbuf_pool.tile([P, 2, S_PAD], f32)

        def invn_group(i):
            """Batched 1/sqrt(ssq) with 2 Newton steps for chunks [i-G+1, i]."""
            g0 = i - (GROUP - 1)
            s_g = ssq[:, g0:i + 1]
            rec = nr_pool.tile([P, GROUP], f32, tag="rec")
            nc.vector.reciprocal(rec[:], s_g)
            y0 = nr_pool.tile([P, GROUP], f32, tag="y0")
            nc.scalar.sqrt(y0[:], rec[:])
            t1 = nr_pool.tile([P, GROUP], f32, tag="t1")
            nc.vector.tensor_tensor(t1[:], y0[:], y0[:], op=mult)
            t2 = nr_pool.tile([P, GROUP], f32, tag="t2")
            nc.vector.tensor_tensor(t2[:], t1[:], s_g, op=mult)
            t3 = nr_pool.tile([P, GROUP], f32, tag="t3")
            nc.vector.tensor_scalar(t3[:], t2[:], -0.5, 1.5, op0=mult, op1=add)
            y1 = nr_pool.tile([P, GROUP], f32, tag="y1")
            nc.vector.tensor_tensor(y1[:], y0[:], t3[:], op=mult)
            t4 = nr_pool.tile([P, GROUP], f32, tag="t4")
            nc.vector.tensor_tensor(t4[:], y1[:], y1[:], op=mult)
            t5 = nr_pool.tile([P, GROUP], f32, tag="t5")
            nc.vector.tensor_tensor(t5[:], t4[:], s_g, op=mult)
            t6 = nr_pool.tile([P, GROUP], f32, tag="t6")
            nc.vector.tensor_scalar(t6[:], t5[:], -0.5, 1.5, op0=mult, op1=add)
            nc.vector.tensor_tensor(invn[:, g0:i + 1], y1[:], t6[:], op=mult)

        wc_tiles = {}
        for i in range(CHUNKS):
            r0, r1 = i * P, (i + 1) * P

            # ---- phase 1 (emitted at group head): load + sumsq ----
            if i % GROUP == 0:
                for j in range(i, i + GROUP):
                    jr0, jr1 = j * P, (j + 1) * P
                    wc_j = wc_pool.tile([P, IN_F], f32, tag="wc", name=f"wc{j}")
                    wc_tiles[j] = wc_j
                    nc.sync.dma_start(wc_j[:], w[jr0:jr1, :])
                    # new_weight shard passthrough
                    nc.sync.dma_start(nw[jr0:jr1, :], wc_j[:])
                    # per-class sum of squares; alternate engines
                    sq = sq_pool.tile([P, IN_F], f32, tag="sq", name=f"sq{j}")
                    if j % 2 == 0:
                        nc.scalar.activation(sq[:], wc_j[:], Square,
                                             accum_out=ssq[:, j:j + 1])
                    else:
                        nc.vector.scalar_tensor_tensor(
                            sq[:], wc_j[:], 1.0, wc_j[:], mult, mult,
                            accum_out=ssq[:, j:j + 1])
                invn_group(i + GROUP - 1)

            wc = wc_tiles.pop(i)

            if variant == "b_fp32":
                # normalize rows while still class-major (per-partition scalar)
                wn = wn_pool.tile([P, IN_F], f32, tag="wn")
                nc.vector.tensor_scalar_mul(wn[:], wc[:], invn[:, i:i + 1])

                # transpose to feat-major via PE: 4x [128c,128f] -> [128f,128c]
                wt_ps = psa_pool.tile([P, IN_F], f32, tag="wt_ps")
                for k in range(4):
                    nc.tensor.matmul(
                        wt_ps[:, k * P:(k + 1) * P], wn[:, k * P:(k + 1) * P],
                        identity[:], is_transpose=True,
                        start=(k == 0), stop=(k == 3),
                    )
                wt = wt_pool.tile([P, IN_F], f32, tag="wt")
                nc.scalar.copy(wt[:], wt_ps[:])

                # out[m, c] = sum_f x^T[f, m] * wn^T[f, c]
                ops = psb_pool.tile([P, BS], f32, tag="ops")
                first = True
                for k in range(4):
                    for h in range(2):
                        nc.tensor.matmul(
                            ops[:, h * P:(h + 1) * P],
                            xt_sb[:, k, h * P:(h + 1) * P],
                            wt[:, k * P:(k + 1) * P],
                            start=first, stop=(k == 3 and h == 1),
                        )
                        first = False
                nc.vector.tensor_copy(
                    out_sb[:, :, r0:r1],
                    ops.rearrange("p (two c) -> p two c", two=2),
                )
            else:  # a_fp32r
                # transpose RAW W chunk; invn applied after the matmul
                wt_ps = psa_pool.tile([P, IN_F], f32, tag="wt_ps")
                for k in range(4):
                    nc.tensor.matmul(
                        wt_ps[:, k * P:(k + 1) * P], wc[:, k * P:(k + 1) * P],
                        identity[:], is_transpose=True,
                        start=(k == 0), stop=(k == 3),
                    )
                wt = wt_pool.tile([P, IN_F], f32r, tag="wt")
                nc.scalar.copy(wt[:], wt_ps[:])
                xt_r = xt_sb.bitcast(f32r)  # pre-rounded on host

                ocp = psb_pool.tile([P, BS], f32, tag="ocp")
                for k in range(4):
                    nc.tensor.matmul(
                        ocp[:], wt[:, k * P:(k + 1) * P], xt_r[:, k, :],
                        start=(k == 0), stop=(k == 3),
                    )
                # scale by 1/||W_c|| while copying PSUM -> SBUF
                oc = oc_pool.tile([P, BS], f32, tag="oc")
                nc.vector.tensor_scalar_mul(oc[:], ocp[:], invn[:, i:i + 1])
                # transpose back to batch-major
                ob_ps = psc_pool.tile([P, BS], f32, tag="ob_ps")
                nc.tensor.matmul(ob_ps[:, 0:P], oc[:, 0:P], identity[:],
                                 is_transpose=True, start=True, stop=False)
                nc.tensor.matmul(ob_ps[:, P:BS], oc[:, P:BS], identity[:],
                                 is_transpose=True, start=False, stop=True)
                nc.scalar.copy(
                    out_sb[:, :, r0:r1],
                    ob_ps.rearrange("p (two c) -> p two c", two=2),
                )

            # flush finished output columns
            if i % OUT_DMA_GROUP == OUT_DMA_GROUP - 1:
                c0 = (i - (OUT_DMA_GROUP - 1)) * P
                c1 = r1
                nc.sync.dma_start(out[0:P, c0:c1], out_sb[:, 0, c0:c1])
                nc.sync.dma_start(out[P:BS, c0:c1], out_sb[:, 1, c0:c1])

    nc.compile()
    return nc


def _get_nc():
    global _COMPILED_NC
    if _COMPILED_NC is None:
        _COMPILED_NC = _build_nc()
    return _COMPILED_NC


def _run_device(w_pad, x_t, trace=False, nc=None, **kw):
    from concourse.bass_utils import run_bass_kernel_spmd

    if nc is None:
        nc = _get_nc()
    in_maps = [
        {"w": np.ascontiguousarray(w_pad[i * S_PAD:(i + 1) * S_PAD]), "xT": x_t}
        for i in range(NCORES)
    ]
    return run_bass_kernel_spmd(
        nc, in_maps, core_ids=list(range(NCORES)), trace=trace, **kw
    )


def _host_corrections(input_f32, weight_f32, label):
    """Margin-term values (per batch row) and scatter-means (per touched row)."""
    x = input_f32.astype(np.float64)
    norms = np.sqrt((x * x).sum(axis=1))                      # ||x||  [256]
    fn = x / np.maximum(norms, EPS)[:, None]
    wrow = weight_f32[label].astype(np.float64)
    wn = wrow / np.maximum(np.sqrt((wrow * wrow).sum(axis=1)), EPS)[:, None]
    cos = np.clip(np.einsum("bf,bf->b", fn, wn), -1.0, 1.0)
    cosm = 8.0 * cos**4 - 8.0 * cos**2 + 1.0
    theta = np.arccos(cos)
    k = np.floor(M_MARGIN * theta / PI_APPROX)
    sign = 1.0 - 2.0 * np.mod(k, 2.0)
    phi = sign * cosm - 2.0 * k
    diag_vals = ((phi - cos) / (1.0 + LAMB) + cos) * norms    # [256]

    uniq, inv_idx, counts = np.unique(
        label, return_inverse=True, return_counts=True
    )
    sums = np.zeros((len(uniq), IN_F), np.float64)
    np.add.at(sums, inv_idx, fn)
    means = (sums / counts[:, None]).astype(np.float32)
    return diag_vals.astype(np.float32), uniq, means


def kernel(input, weight, label):
    input = np.ascontiguousarray(np.asarray(input), dtype=np.float32)
    weight = np.ascontiguousarray(np.asarray(weight), dtype=np.float32)
    label = np.asarray(label).astype(np.int64)

    # shard + pad weight along classes; pad rows are benign ones
    w_pad = np.empty((NCORES * S_PAD, IN_F), np.float32)
    w_pad[:OUT_F] = weight
    w_pad[OUT_F:] = 1.0
    x_t = np.ascontiguousarray(input.T)

    res = _run_device(w_pad, x_t)

    output = np.empty((BS, OUT_F), np.float32)
    new_weight = np.empty((OUT_F, IN_F), np.float32)
    for i in range(NCORES):
        lo = i * S_PAD
        hi = min(lo + S_PAD, OUT_F)
        nvalid = hi - lo
        output[:, lo:hi] = res.results[i]["out"][:, :nvalid]
        new_weight[lo:hi] = res.results[i]["nw"][:nvalid]

    diag_vals, uniq, means = _host_corrections(input, weight, label)
    output[np.arange(BS), label] = diag_vals
    new_weight[uniq] = means
    return output, new_weight


# revision 15
# speedup vs baseline: 1.2353x; 1.2353x over previous
"""Trainium2 Bass kernel for nn_DWIAngleLinear (A-Softmax / vq_codebook).

Math (see reference):
  out      = (one_hot*(phi-cos)/(1+lamb) + cos) * ||x||    [256, 85742]
  new_w    = scatter-mean(f_norm, label) over touched rows  [85742, 512]

Key restructuring for the device:
  * Bulk of `out` is cos_theta * ||x||  =  x @ (W / ||W||_rows).T  -- using the
    UNNORMALIZED input folds the ||x|| row scale into the matmul for free.
  * The one-hot margin term touches only 256 elements -> host fixup.
  * new_weight differs from weight in <=256 rows -> device streams the weight
    shard back out; host patches the touched rows with the scatter-means.

Sharding: classes (out_features) split across 8 cores, padded to 8 x 10752.
Each core: W shard [10752, 512] + x^T [512, 256] -> out shard [256, 10752]
and new_weight shard [10752, 512].

Per 128-class chunk on each core:
  DMA W chunk (class-major) -> sumsq (ACT/DVE alternating) -> 1/||w|| via
  DVE-reciprocal + ACT-sqrt + 2 Newton steps (batched per 12 chunks) ->
  normalize (DVE tensor_scalar) -> PE-transpose to feat-major -> 8 fp32
  matmuls vs resident x^T -> PSUM->SBUF copy into a staged [256, S] buffer
  -> bulk DMA out.  W chunk also DMA'd back out as the new_weight shard.
"""

import numpy as np

BS = 256
IN_F = 512
OUT_F = 85742
NCORES = 8
P = 128
S_PAD = 10752            # classes per core, = CHUNKS * P
CHUNKS = 84
GROUP = 6                # chunks per rsqrt batch
OUT_DMA_GROUP = 7        # chunks per output-column DMA flush (divides CHUNKS)
# group partition: small first groups so the PE pipeline starts early
GROUP_SIZES = [2, 4] + [GROUP] * ((CHUNKS - 6) // GROUP)
assert sum(GROUP_SIZES) == CHUNKS

M_MARGIN = 4
PI_APPROX = 3.14159265
LAMB = max(5.0, 1000.0 * (1.0 + 0.12 * 1) ** (-1))
EPS = 1e-12

_COMPILED_NC = None


def _build_nc(variant="b_fp32"):
    """Build + compile the per-core Bass/Tile kernel (label independent).

    variant:
      "b_fp32"  - batch-major matmul output, full fp32 matmuls (4 cyc/row)
      "a_fp32r" - class-major matmul output (N=256 moving dim), TF32 matmuls
    """
    from contextlib import ExitStack

    import concourse.bacc as bacc
    import concourse.mybir as mybir
    import concourse.tile as tile
    from concourse.masks import make_identity

    f32 = mybir.dt.float32
    f32r = mybir.dt.float32r
    mult = mybir.AluOpType.mult
    add = mybir.AluOpType.add
    subtract = mybir.AluOpType.subtract
    Square = mybir.ActivationFunctionType.Square

    nc = bacc.Bacc("TRN2", target_bir_lowering=False, debug=False)

    xt_dt = f32r if variant == "a_fp32r" else f32
    w = nc.dram_tensor("w", [S_PAD, IN_F], f32, kind="ExternalInput").ap()
    xT = nc.dram_tensor("xT", [IN_F, BS], xt_dt, kind="ExternalInput").ap()
    out = nc.dram_tensor("out", [BS, S_PAD], f32, kind="ExternalOutput").ap()
    nw = nc.dram_tensor("nw", [S_PAD, IN_F], f32, kind="ExternalOutput").ap()

    with tile.TileContext(nc) as tc, ExitStack() as ctx:
        const_pool = ctx.enter_context(tc.tile_pool(name="const", bufs=1))
        outbuf_pool = ctx.enter_context(tc.tile_pool(name="outbuf", bufs=1))
        wc_pool = ctx.enter_context(tc.tile_pool(name="wc", bufs=2 * GROUP + 2))
        wn_pool = ctx.enter_context(tc.tile_pool(name="wn", bufs=3))
        sq_pool = ctx.enter_context(tc.tile_pool(name="sq", bufs=4))
        wt_pool = ctx.enter_context(tc.tile_pool(name="wt", bufs=3))
        nr_pool = ctx.enter_context(tc.tile_pool(name="nr", bufs=2))
        n_psa, n_psb = (3, 4) if variant == "b_fp32" else (2, 3)
        psa_pool = ctx.enter_context(tc.tile_pool(name="psa", bufs=n_psa, space="PSUM"))
        psb_pool = ctx.enter_context(tc.tile_pool(name="psb", bufs=n_psb, space="PSUM"))
        if variant == "a_fp32r":
            oc_pool = ctx.enter_context(tc.tile_pool(name="oc", bufs=3))
            psc_pool = ctx.enter_context(tc.tile_pool(name="psc", bufs=2, space="PSUM"))

        identity = const_pool.tile([P, P], f32)
        make_identity(nc, identity)

        # x^T staged as [128 feat, 4 kchunk, 256 batch]
        xt_sb = const_pool.tile([P, 4, BS], xt_dt)
        nc.sync.dma_start(xt_sb[:], xT.rearrange("(k p) m -> p k m", p=P))

        ssq = const_pool.tile([P, CHUNKS], f32)   # per-class sum(W^2)
        invn = const_pool.tile([P, CHUNKS], f32)  # per-class 1/||W||

        # final batch-major output staged in SBUF: [128, 2 (m-half), S_PAD]
        out_sb = outbuf_pool.tile([P, 2, S_PAD], f32)

        def invn_group(g0, g1):
            """Batched 1/sqrt(ssq) with 2 Newton steps for chunks [g0, g1)."""
            n = g1 - g0
            s_g = ssq[:, g0:g1]
            rec = nr_pool.tile([P, n], f32, tag="rec", padded_shape=[P, GROUP])
            nc.vector.reciprocal(rec[:], s_g)
            y0 = nr_pool.tile([P, n], f32, tag="y0", padded_shape=[P, GROUP])
            nc.scalar.sqrt(y0[:], rec[:])
            t1 = nr_pool.tile([P, n], f32, tag="t1", padded_shape=[P, GROUP])
            nc.vector.tensor_tensor(t1[:], y0[:], y0[:], op=mult)
            t2 = nr_pool.tile([P, n], f32, tag="t2", padded_shape=[P, GROUP])
            nc.vector.tensor_tensor(t2[:], t1[:], s_g, op=mult)
            t3 = nr_pool.tile([P, n], f32, tag="t3", padded_shape=[P, GROUP])
            nc.vector.tensor_scalar(t3[:], t2[:], -0.5, 1.5, op0=mult, op1=add)
            y1 = nr_pool.tile([P, n], f32, tag="y1", padded_shape=[P, GROUP])
            nc.vector.tensor_tensor(y1[:], y0[:], t3[:], op=mult)
            t4 = nr_pool.tile([P, n], f32, tag="t4", padded_shape=[P, GROUP])
            nc.vector.tensor_tensor(t4[:], y1[:], y1[:], op=mult)
            t5 = nr_pool.tile([P, n], f32, tag="t5", padded_shape=[P, GROUP])
            nc.vector.tensor_tensor(t5[:], t4[:], s_g, op=mult)
            t6 = nr_pool.tile([P, n], f32, tag="t6", padded_shape=[P, GROUP])
            nc.vector.tensor_scalar(t6[:], t5[:], -0.5, 1.5, op0=mult, op1=add)
            nc.vector.tensor_tensor(invn[:, g0:g1], y1[:], t6[:], op=mult)

        group_head = {}
        _start = 0
        for _sz in GROUP_SIZES:
            group_head[_start] = _sz
            _start += _sz

        wc_tiles = {}
        for i in range(CHUNKS):
            r0, r1 = i * P, (i + 1) * P

            # ---- phase 1 (emitted at group head): load + sumsq ----
            if i in group_head:
                gsz = group_head[i]
                for j in range(i, i + gsz):
                    jr0, jr1 = j * P, (j + 1) * P
                    wc_j = wc_pool.tile([P, IN_F], f32, tag="wc", name=f"wc{j}")
                    wc_tiles[j] = wc_j
                    nc.sync.dma_start(wc_j[:], w[jr0:jr1, :])
                    # new_weight shard passthrough
                    nc.sync.dma_start(nw[jr0:jr1, :], wc_j[:])
                    # per-class sum of squares; alternate engines
                    sq = sq_pool.tile([P, IN_F], f32, tag="sq", name=f"sq{j}")
                    if j % 2 == 0:
                        nc.scalar.activation(sq[:], wc_j[:], Square,
                                             accum_out=ssq[:, j:j + 1])
                    else:
                        nc.vector.scalar_tensor_tensor(
                            sq[:], wc_j[:], 1.0, wc_j[:], mult, mult,
                            accum_out=ssq[:, j:j + 1])
                invn_group(i, i + gsz)

            wc = wc_tiles.pop(i)

            if variant == "b_fp32":
                # normalize rows while still class-major (per-partition scalar)
                wn = wn_pool.tile([P, IN_F], f32, tag="wn")
                nc.vector.tensor_scalar_mul(wn[:], wc[:], invn[:, i:i + 1])

                # transpose to feat-major via PE: 4x [128c,128f] -> [128f,128c]
                wt_ps = psa_pool.tile([P, IN_F], f32, tag="wt_ps")
                for k in range(4):
                    nc.tensor.matmul(
                        wt_ps[:, k * P:(k + 1) * P], wn[:, k * P:(k + 1) * P],
                        identity[:], is_transpose=True,
                        start=(k == 0), stop=(k == 3),
                    )
                wt = wt_pool.tile([P, IN_F], f32, tag="wt")
                nc.scalar.copy(wt[:], wt_ps[:])

                # out[m, c] = sum_f x^T[f, m] * wn^T[f, c]
                ops = psb_pool.tile([P, BS], f32, tag="ops")
                first = True
                for k in range(4):
                    for h in range(2):
                        nc.tensor.matmul(
                            ops[:, h * P:(h + 1) * P],
                            xt_sb[:, k, h * P:(h + 1) * P],
                            wt[:, k * P:(k + 1) * P],
                            start=first, stop=(k == 3 and h == 1),
                        )
                        first = False
                nc.vector.tensor_copy(
                    out_sb[:, :, r0:r1],
                    ops.rearrange("p (two c) -> p two c", two=2),
                )
            else:  # a_fp32r
                # transpose RAW W chunk; invn applied after the matmul
                wt_ps = psa_pool.tile([P, IN_F], f32, tag="wt_ps")
                for k in range(4):
                    nc.tensor.matmul(
                        wt_ps[:, k * P:(k + 1) * P], wc[:, k * P:(k + 1) * P],
                        identity[:], is_transpose=True,
                        start=(k == 0), stop=(k == 3),
                    )
                wt = wt_pool.tile([P, IN_F], f32r, tag="wt")
                nc.scalar.copy(wt[:], wt_ps[:])
                xt_r = xt_sb  # dram dtype is f32r; host supplies pre-rounded bits

                ocp = psb_pool.tile([P, BS], f32, tag="ocp")
                for k in range(4):
                    nc.tensor.matmul(
                        ocp[:], wt[:, k * P:(k + 1) * P], xt_r[:, k, :],
                        start=(k == 0), stop=(k == 3),
                    )
                # scale by 1/||W_c|| while copying PSUM -> SBUF
                oc = oc_pool.tile([P, BS], f32, tag="oc")
                nc.vector.tensor_scalar_mul(oc[:], ocp[:], invn[:, i:i + 1])
                # transpose back to batch-major
                ob_ps = psc_pool.tile([P, BS], f32, tag="ob_ps")
                nc.tensor.matmul(ob_ps[:, 0:P], oc[:, 0:P], identity[:],
                                 is_transpose=True, start=True, stop=False)
                nc.tensor.matmul(ob_ps[:, P:BS], oc[:, P:BS], identity[:],
                                 is_transpose=True, start=False, stop=True)
                nc.scalar.copy(
                    out_sb[:, :, r0:r1],
                    ob_ps.rearrange("p (two c) -> p two c", two=2),
                )

            # flush finished output columns
            if i % OUT_DMA_GROUP == OUT_DMA_GROUP - 1:
                c0 = (i - (OUT_DMA_GROUP - 1)) * P
                c1 = r1
                nc.sync.dma_start(out[0:P, c0:c1], out_sb[:, 0, c0:c1])
                nc.sync.dma_start(out[P:BS, c0:c1], out_sb[:, 1, c0:c1])

    nc.compile()
    return nc


def _get_nc():
    global _COMPILED_NC
    if _COMPILED_NC is None:
        _COMPILED_NC = _build_nc()
    return _COMPILED_NC


def tf32_round(x):
    """Round fp32 array to TF32 (10-bit mantissa, round-to-nearest-even)."""
    u = np.ascontiguousarray(x, np.float32).view(np.uint32)
    r = (u + 0x00000FFF + ((u >> 13) & 1)) & np.uint32(0xFFFFE000)
    return r.view(np.float32)


def _run_device(w_pad, x_t, trace=False, nc=None, **kw):
    from concourse.bass_utils import run_bass_kernel_spmd

    if nc is None:
        nc = _get_nc()
    in_maps = [
        {"w": np.ascontiguousarray(w_pad[i * S_PAD:(i + 1) * S_PAD]), "xT": x_t}
        for i in range(NCORES)
    ]
    return run_bass_kernel_spmd(
        nc, in_maps, core_ids=list(range(NCORES)), trace=trace, **kw
    )


def _host_corrections(input_f32, weight_f32, label):
    """Margin-term values (per batch row) and scatter-means (per touched row)."""
    x = input_f32.astype(np.float64)
    norms = np.sqrt((x * x).sum(axis=1))                      # ||x||  [256]
    fn = x / np.maximum(norms, EPS)[:, None]
    wrow = weight_f32[label].astype(np.float64)
    wn = wrow / np.maximum(np.sqrt((wrow * wrow).sum(axis=1)), EPS)[:, None]
    cos = np.clip(np.einsum("bf,bf->b", fn, wn), -1.0, 1.0)
    cosm = 8.0 * cos**4 - 8.0 * cos**2 + 1.0
    theta = np.arccos(cos)
    k = np.floor(M_MARGIN * theta / PI_APPROX)
    sign = 1.0 - 2.0 * np.mod(k, 2.0)
    phi = sign * cosm - 2.0 * k
    diag_vals = ((phi - cos) / (1.0 + LAMB) + cos) * norms    # [256]

    uniq, inv_idx, counts = np.unique(
        label, return_inverse=True, return_counts=True
    )
    sums = np.zeros((len(uniq), IN_F), np.float64)
    np.add.at(sums, inv_idx, fn)
    means = (sums / counts[:, None]).astype(np.float32)
    return diag_vals.astype(np.float32), uniq, means


def kernel(input, weight, label):
    input = np.ascontiguousarray(np.asarray(input), dtype=np.float32)
    weight = np.ascontiguousarray(np.asarray(weight), dtype=np.float32)
    label = np.asarray(label).astype(np.int64)

    # shard + pad weight along classes; pad rows are benign ones
    w_pad = np.empty((NCORES * S_PAD, IN_F), np.float32)
    w_pad[:OUT_F] = weight
    w_pad[OUT_F:] = 1.0
    x_t = np.ascontiguousarray(input.T)

    res = _run_device(w_pad, x_t)

    output = np.empty((BS, OUT_F), np.float32)
    new_weight = np.empty((OUT_F, IN_F), np.float32)
    for i in range(NCORES):
        lo = i * S_PAD
        hi = min(lo + S_PAD, OUT_F)
        nvalid = hi - lo
        output[:, lo:hi] = res.results[i]["out"][:, :nvalid]
        new_weight[lo:hi] = res.results[i]["nw"][:nvalid]

    diag_vals, uniq, means = _host_corrections(input, weight, label)
    output[np.arange(BS), label] = diag_vals
    new_weight[uniq] = means
    return output, new_weight
